# revision 7
# baseline (speedup 1.0000x reference)
"""Trainium2 Bass kernel for nn_CoPredictor (biaffine co-predictor).

Math (per batch b, class k):
    h    = gelu(x_b @ mlp1_w.T + b1)          (512,256)
    t    = gelu(y_b @ mlp2_w.T + b2)          (512,256)
    head = lrelu(x_b @ head_w.T + bh, .01)    (512,256)
    tail = lrelu(x_b @ tail_w.T + bt, .01)    (512,256)
    out[b,k,m,n] = (h A_k t^T)[m,n] + r[k,m] + cv[k,n] + T0[k, clip(n-m+15,0,29)]
  with A_k = biaf_W[k,:256,:256],
       r[k,m]  = h[m].u_k + head[m].Wh_k          (u_k = biaf_W[k,:256,256], Wh_k = W[k,:256])
       cv[k,n] = t[n].v_k + tail[n].Wt_k          (v_k = biaf_W[k,256,:256], Wt_k = W[k,257:513])
       T0[k,d] = size_emb[d].Ws_k + W[k,256] + W[k,513] + biaf_W[k,256,256]
(z is unused by the reference.)

Sharding: 8 cores = batch(2) x m-half(2) x class-half(2x7).  Each core computes
out[b, k0:k0+7, m0:m0+512/2, :].  All device matmuls run in float32r (TF32-class).
The n axis is processed reversed (n' = 511-n) on device so the Toeplitz T0 term
becomes a function of (p + n') and can be materialized by a single overlapping
DMA from a small table; the host flips n back at the end.
"""
import sys

sys.path.insert(0, "/opt/trn_rl_repo")

import numpy as np

B, N, HID = 2, 512, 768
BIAF, CLS = 256, 14
KH = 7          # classes per core
P = 128

_nc = None


def _build_program(act_mode="hw"):
    import concourse.bass as bass
    import concourse.bacc as bacc
    import concourse.mybir as mybir
    import concourse.tile as tile

    F32 = mybir.dt.float32
    F32R = mybir.dt.float32r
    if act_mode == "hw":
        GELU = mybir.ActivationFunctionType.Gelu
        LRELU = mybir.ActivationFunctionType.Prelu   # parametric_relu, same table set as gelu
    else:  # CoreSim doesn't implement Gelu/Prelu; substitute for structure validation
        GELU = mybir.ActivationFunctionType.Tanh
        LRELU = mybir.ActivationFunctionType.Relu

    nc = bacc.Bacc("TRN2", target_bir_lowering=False, debug=False, num_devices=8)

    # ---- per-core inputs (all float32 bytes; f32r tensors feed matmuls) ----
    wm = nc.dram_tensor("wm", [4, HID, BIAF], F32R, kind="ExternalInput").ap()
    xtc = nc.dram_tensor("xtc", [HID, 256], F32R, kind="ExternalInput").ap()
    ytr = nc.dram_tensor("ytr", [HID, N], F32R, kind="ExternalInput").ap()
    xtr = nc.dram_tensor("xtr", [HID, N], F32R, kind="ExternalInput").ap()
    abig = nc.dram_tensor("abig", [KH, BIAF, BIAF], F32R, kind="ExternalInput").ap()
    uvw = nc.dram_tensor("uvw", [4, BIAF, KH], F32R, kind="ExternalInput").ap()
    biasv = nc.dram_tensor("biasv", [4, BIAF], F32, kind="ExternalInput").ap()
    hh = nc.dram_tensor("hh", [KH, 767], F32, kind="ExternalInput").ap()
    out_d = nc.dram_tensor("out", [KH, 2, P, N], F32, kind="ExternalOutput").ap()

    with tile.TileContext(nc) as tc:
        with tc.tile_pool(name="const", bufs=1) as cp, \
             tc.tile_pool(name="work", bufs=3) as wp, \
             tc.tile_pool(name="dram", bufs=1, space="DRAM") as dp, \
             tc.tile_pool(name="ps2", bufs=2, space="PSUM") as ps2, \
             tc.tile_pool(name="psm", bufs=2, space="PSUM") as psm:

            # ---- resident constants / activations ----
            wm_sb = cp.tile([P, 4 * 6 * 256], F32R)       # ((wi*6+dc)*256 + j)
            nc.sync.dma_start(
                out=wm_sb[:, :].rearrange("p (w dc j) -> p w dc j", w=4, dc=6),
                in_=wm.rearrange("w (dc p) j -> p w dc j", p=P))
            xtc_sb = cp.tile([P, 6 * 256], F32R)          # (dc*256 + m)
            nc.sync.dma_start(
                out=xtc_sb[:, :].rearrange("p (dc m) -> p dc m", dc=6),
                in_=xtc.rearrange("(dc p) m -> p dc m", p=P))
            ytr_sb = cp.tile([P, 6 * 512], F32R)
            nc.sync.dma_start(
                out=ytr_sb[:, :].rearrange("p (dc n) -> p dc n", dc=6),
                in_=ytr.rearrange("(dc p) n -> p dc n", p=P))
            xtr_sb = cp.tile([P, 6 * 512], F32R)
            nc.sync.dma_start(
                out=xtr_sb[:, :].rearrange("p (dc n) -> p dc n", dc=6),
                in_=xtr.rearrange("(dc p) n -> p dc n", p=P))
            a_sb = cp.tile([P, KH * 2 * 2 * 128], F32R)   # (((kk*2+ic)*2+jh)*128 + c)
            nc.sync.dma_start(
                out=a_sb[:, :].rearrange("p (k ic jh c) -> p k ic jh c", k=KH, ic=2, jh=2),
                in_=abig.rearrange("k (ic p) (jh c) -> p k ic jh c", p=P, c=P))
            uvw_sb = cp.tile([P, 4 * 2 * KH], F32R)       # ((wi*2+jc)*7 + kk)
            nc.sync.dma_start(
                out=uvw_sb[:, :].rearrange("p (w jc k) -> p w jc k", w=4, jc=2),
                in_=uvw.rearrange("w (jc p) k -> p w jc k", p=P))
            bias_sb = cp.tile([P, 8], F32)                # (wi*2 + jc)
            nc.sync.dma_start(
                out=bias_sb[:, :].rearrange("p (w jc) -> p w jc", w=4),
                in_=biasv.rearrange("w (jc p) -> p w jc", p=P))

            hT_sb = cp.tile([P, 2 * 256], F32R)           # (jc*256 + m)
            tT_sb = cp.tile([P, 2 * 512], F32R)           # (jc*512 + n')
            headT_sb = cp.tile([P, 2 * 256], F32R)
            tailT_sb = cp.tile([P, 2 * 512], F32R)

            # ---- stage A: the four MLPs, transposed layout (features on partitions) ----
            def mlp(wi, rhs_sb, nfree, dst_sb, func, alpha):
                for jc in range(2):
                    pt = ps2.tile([P, 512], mybir.dt.float32, tag="mlp")
                    for dc in range(6):
                        nc.tensor.matmul(
                            pt[:, :nfree],
                            wm_sb[:, (wi * 6 + dc) * 256 + jc * 128:(wi * 6 + dc) * 256 + (jc + 1) * 128],
                            rhs_sb[:, dc * nfree:(dc + 1) * nfree],
                            start=(dc == 0), stop=(dc == 5))
                    nc.scalar.activation(
                        dst_sb[:, jc * nfree:(jc + 1) * nfree], pt[:, :nfree],
                        func, bias=bias_sb[:, wi * 2 + jc:wi * 2 + jc + 1], alpha=alpha)

            mlp(0, xtc_sb, 256, hT_sb, GELU, 0.0)
            mlp(1, ytr_sb, 512, tT_sb, GELU, 0.0)
            mlp(2, xtc_sb, 256, headT_sb, LRELU, 0.01)
            mlp(3, xtr_sb, 512, tailT_sb, LRELU, 0.01)

            # ---- stage B: rank-1 row/col terms ----
            # Rt[kk, m] = h.u + head.Wh   (7,256);  Cv[kk, n'] = t.v + tail.Wt  (7,512)
            rt_ps = psm.tile([KH, 256], mybir.dt.float32, tag="rc")
            steps = [(0, hT_sb, 256), (1, headT_sb, 256)]
            for si, (ui, src, nf) in enumerate(steps):
                for jc in range(2):
                    nc.tensor.matmul(
                        rt_ps[:, :],
                        uvw_sb[:, (ui * 2 + jc) * KH:(ui * 2 + jc + 1) * KH],
                        src[:, jc * nf:(jc + 1) * nf],
                        start=(si == 0 and jc == 0), stop=(si == 1 and jc == 1))
            cv_ps = psm.tile([KH, 512], mybir.dt.float32, tag="rc")
            steps = [(2, tT_sb, 512), (3, tailT_sb, 512)]
            for si, (ui, src, nf) in enumerate(steps):
                for jc in range(2):
                    nc.tensor.matmul(
                        cv_ps[:, :],
                        uvw_sb[:, (ui * 2 + jc) * KH:(ui * 2 + jc + 1) * KH],
                        src[:, jc * nf:(jc + 1) * nf],
                        start=(si == 0 and jc == 0), stop=(si == 1 and jc == 1))
            rt_sb = wp.tile([KH, 256], F32R, tag="rt")
            cv_sb = wp.tile([KH, 512], F32R, tag="cv")
            nc.vector.tensor_copy(rt_sb[:, :], rt_ps[:, :])
            nc.vector.tensor_copy(cv_sb[:, :], cv_ps[:, :])

            # round-trip through DRAM to re-partition (7,X) rows into 2-row packs
            ones_sb = cp.tile([1, KH * 512], F32R)
            nc.vector.memset(ones_sb[:, :].bitcast(F32), 1.0)
            scr_r = dp.tile([2, KH * 256], F32R)
            scr_c = dp.tile([2, KH * 512], F32R)
            nc.sync.dma_start(out=scr_r[0:1, :], in_=ones_sb[:, :KH * 256])
            nc.sync.dma_start(out=scr_r[1:2, :], in_=rt_sb[:, :])
            nc.sync.dma_start(out=scr_c[0:1, :], in_=cv_sb[:, :])
            nc.sync.dma_start(out=scr_c[1:2, :], in_=ones_sb[:, :])
            rall = cp.tile([2, KH * 256], F32R)    # row0 = ones, row1 = r
            cvall = cp.tile([2, KH * 512], F32R)   # row0 = cv,  row1 = ones
            nc.sync.dma_start(out=rall[0:2, :], in_=scr_r[:, :])
            nc.sync.dma_start(out=cvall[0:2, :], in_=scr_c[:, :])

            # ---- stage C: per-class biaffine ----
            import concourse.bass as bass_mod
            for kk in range(KH):
                # G_k^T (j=256, m=256) packed as (128, [jh*256+m])
                g_ps = ps2.tile([P, 512], mybir.dt.float32, tag="g")
                for jh in range(2):
                    for ic in range(2):
                        nc.tensor.matmul(
                            g_ps[:, jh * 256:(jh + 1) * 256],
                            a_sb[:, ((kk * 2 + ic) * 2 + jh) * 128:((kk * 2 + ic) * 2 + jh + 1) * 128],
                            hT_sb[:, ic * 256:(ic + 1) * 256],
                            start=(ic == 0), stop=(ic == 1))
                g_sb = wp.tile([P, 512], F32R, tag="gsb")
                nc.vector.tensor_copy(g_sb[:, :], g_ps[:, :])
                for mt in range(2):
                    m_ps = psm.tile([P, 512], mybir.dt.float32, tag="m")
                    for jh in range(2):
                        nc.tensor.matmul(
                            m_ps[:, :],
                            g_sb[:, jh * 256 + mt * 128:jh * 256 + (mt + 1) * 128],
                            tT_sb[:, jh * 512:(jh + 1) * 512],
                            start=(jh == 0), stop=False)
                    nc.tensor.matmul(
                        m_ps[:, :],
                        rall[:, kk * 256 + mt * 128:kk * 256 + (mt + 1) * 128],
                        cvall[:, kk * 512:(kk + 1) * 512],
                        start=False, stop=True)
                    tl = wp.tile([P, 512], F32, tag="tl")
                    hap = bass_mod.AP(tensor=hh.tensor, offset=kk * 767 + mt * 128,
                                      ap=[[1, P], [1, 512]])
                    nc.sync.dma_start(out=tl[:, :], in_=hap)
                    osb = wp.tile([P, 512], F32, tag="osb")
                    nc.vector.tensor_tensor(osb[:, :], m_ps[:, :], tl[:, :],
                                            mybir.AluOpType.add)
                    nc.sync.dma_start(out=out_d[kk, mt], in_=osb[:, :])

    nc.compile()
    return nc


def _get_program():
    global _nc
    if _nc is None:
        _nc = _build_program()
    return _nc


def make_in_maps(x, y, mlp1_w, mlp1_b, mlp2_w, mlp2_b, head_w, head_b,
                 tail_w, tail_b, biaf_W, W, size_emb):
    f = np.float32
    x = np.asarray(x, f)
    y = np.asarray(y, f)
    wm_all = np.ascontiguousarray(
        np.stack([mlp1_w.T, mlp2_w.T, head_w.T, tail_w.T]).astype(f))  # (4,768,256)
    biasv = np.ascontiguousarray(
        np.stack([mlp1_b, mlp2_b, head_b, tail_b]).astype(f))          # (4,256)

    Ws = W[:, 514:539]
    cval = W[:, 256] + W[:, 513] + biaf_W[:, 256, 256]                 # (14,)
    T0 = (size_emb @ Ws.T).T + cval[:, None]                           # (14,30)

    xT = {b: np.ascontiguousarray(x[b].T) for b in range(B)}           # (768,512)
    xTr = {b: np.ascontiguousarray(x[b].T[:, ::-1]) for b in range(B)}
    yTr = {b: np.ascontiguousarray(y[b].T[:, ::-1]) for b in range(B)}

    in_maps = []
    for c in range(8):
        b, mh, khalf = c // 4, (c // 2) % 2, c % 2
        m0, k0 = mh * 256, khalf * KH
        ks = slice(k0, k0 + KH)
        uvw_m = np.ascontiguousarray(np.stack([
            biaf_W[ks, :256, 256].T,     # U    (256,7)
            W[ks, :256].T,               # WhT
            biaf_W[ks, 256, :256].T,     # V
            W[ks, 257:513].T,            # WtT
        ]).astype(f))                                                   # (4,256,7)
        wprime = np.arange(767)
        hh_m = np.ascontiguousarray(np.stack(
            [T0[k0 + kk][np.clip(526 - (wprime + m0), 0, 29)] for kk in range(KH)]
        ).astype(f))                                                    # (7,767)
        in_maps.append({
            "wm": wm_all,
            "xtc": np.ascontiguousarray(xT[b][:, m0:m0 + 256]),
            "ytr": yTr[b],
            "xtr": xTr[b],
            "abig": np.ascontiguousarray(biaf_W[ks, :256, :256].astype(f)),
            "uvw": uvw_m,
            "biasv": biasv,
            "hh": hh_m,
        })
    return in_maps


def assemble(results):
    out = np.empty((B, CLS, N, N), np.float32)
    for c, r in enumerate(results):
        b, mh, khalf = c // 4, (c // 2) % 2, c % 2
        blk = r["out"].reshape(KH, 256, 512)[:, :, ::-1]   # un-flip n
        out[b, khalf * KH:(khalf + 1) * KH, mh * 256:(mh + 1) * 256, :] = blk
    return out


def kernel(**inputs):
    from concourse import bass_utils
    nc = _get_program()
    in_maps = make_in_maps(
        inputs["x"], inputs["y"],
        inputs["mlp1_w"], inputs["mlp1_b"], inputs["mlp2_w"], inputs["mlp2_b"],
        inputs["head_w"], inputs["head_b"], inputs["tail_w"], inputs["tail_b"],
        inputs["biaf_W"], inputs["W"], inputs["size_emb"])
    res = bass_utils.run_bass_kernel_spmd(nc, in_maps, core_ids=list(range(8)),
                                          trace=False)
    return assemble(res.results)


# revision 9
# speedup vs baseline: 3019.1096x; 3019.1096x over previous
"""Trainium2 Bass kernel for nn_CoPredictor (biaffine co-predictor).

Math (per batch b, class k):
    h    = gelu(x_b @ mlp1_w.T + b1)          (512,256)
    t    = gelu(y_b @ mlp2_w.T + b2)          (512,256)
    head = lrelu(x_b @ head_w.T + bh, .01)    (512,256)
    tail = lrelu(x_b @ tail_w.T + bt, .01)    (512,256)
    out[b,k,m,n] = (h A_k t^T)[m,n] + r[k,m] + cv[k,n] + T0[k, clip(n-m+15,0,29)]
  with A_k = biaf_W[k,:256,:256],
       r[k,m]  = h[m].u_k + head[m].Wh_k          (u_k = biaf_W[k,:256,256], Wh_k = W[k,:256])
       cv[k,n] = t[n].v_k + tail[n].Wt_k          (v_k = biaf_W[k,256,:256], Wt_k = W[k,257:513])
       T0[k,d] = size_emb[d].Ws_k + W[k,256] + W[k,513] + biaf_W[k,256,256]
(z is unused by the reference.)

Sharding: 8 cores = batch(2) x m-half(2) x class-half(2x7).  Each core computes
out[b, k0:k0+7, m0:m0+256, :].  Matmuls run in float32r (TF32-class) by default.
The n axis is processed reversed (n' = 511-n) on device so the Toeplitz T0 term
becomes a function of (p + n'), materialized by one overlapping (128,640) DMA
per class from a small bf16 table and accumulated into PSUM via an identity
matmul; the host flips n back at the end.  r/cv broadcast terms ride a K=2
matmul whose 2-row operands are packed via a DRAM round-trip.
"""
import sys

sys.path.insert(0, "/opt/trn_rl_repo")

import numpy as np

B, N, HID = 2, 512, 768
BIAF, CLS = 256, 14
KH = 7          # classes per core
P = 128

MLP_DT = "f32r"    # dtype of the MLP-stage matmuls (weights + x/y activations)
MAIN_DT = "f32r"   # dtype of the biaffine-stage matmuls

_nc = {}


def _build_program(act_mode="hw", reps=0, mlp_dt=None, main_dt=None):
    import concourse.bass as bass
    import concourse.bacc as bacc
    import concourse.mybir as mybir
    import concourse.tile as tile

    mlp_dt = mlp_dt or MLP_DT
    main_dt = main_dt or MAIN_DT
    F32 = mybir.dt.float32
    BF16 = mybir.dt.bfloat16
    DT_MLP = mybir.dt.float32r if mlp_dt == "f32r" else BF16
    DT_MAIN = mybir.dt.float32r if main_dt == "f32r" else BF16
    if act_mode == "hw":
        GELU = mybir.ActivationFunctionType.Gelu
        LRELU = mybir.ActivationFunctionType.Prelu   # parametric_relu, same table set as gelu
    else:  # CoreSim doesn't implement Gelu/Prelu; substitute for structure validation
        GELU = mybir.ActivationFunctionType.Tanh
        LRELU = mybir.ActivationFunctionType.Relu

    nc = bacc.Bacc("TRN2", target_bir_lowering=False, debug=False, num_devices=8)

    wm = nc.dram_tensor("wm", [4, HID, BIAF], DT_MLP, kind="ExternalInput").ap()
    xtc = nc.dram_tensor("xtc", [HID, 256], DT_MLP, kind="ExternalInput").ap()
    ytr = nc.dram_tensor("ytr", [HID, N], DT_MLP, kind="ExternalInput").ap()
    xtr = nc.dram_tensor("xtr", [HID, N], DT_MLP, kind="ExternalInput").ap()
    abig = nc.dram_tensor("abig", [KH, BIAF, BIAF], DT_MAIN, kind="ExternalInput").ap()
    uvw = nc.dram_tensor("uvw", [4, BIAF, KH], DT_MAIN, kind="ExternalInput").ap()
    biasv = nc.dram_tensor("biasv", [4, BIAF], F32, kind="ExternalInput").ap()
    hh = nc.dram_tensor("hh", [KH, 767], BF16, kind="ExternalInput").ap()
    ident = nc.dram_tensor("ident", [P, P], BF16, kind="ExternalInput").ap()
    onesr = nc.dram_tensor("onesr", [1, KH * 512], DT_MAIN, kind="ExternalInput").ap()
    out_d = nc.dram_tensor("out", [KH, 2, P, N], F32, kind="ExternalOutput").ap()

    with tile.TileContext(nc) as tc:
        with tc.tile_pool(name="const", bufs=1) as cp, \
             tc.tile_pool(name="work", bufs=3) as wp, \
             tc.tile_pool(name="dram", bufs=1, space="DRAM") as dp, \
             tc.tile_pool(name="psa", bufs=2, space="PSUM") as psa, \
             tc.tile_pool(name="psm", bufs=4, space="PSUM") as psm:

            def body(_iv=None):
                # ---- resident constants / activations ----
                xtc_sb = cp.tile([P, 6 * 256], DT_MLP, tag="xtc")     # (dc*256 + m)
                nc.sync.dma_start(
                    out=xtc_sb[:, :].rearrange("p (dc m) -> p dc m", dc=6),
                    in_=xtc.rearrange("(dc p) m -> p dc m", p=P))
                wm_sb = cp.tile([P, 4 * 6 * 256], DT_MLP, tag="wm")   # ((wi*6+dc)*256 + j)
                for wi in range(4):
                    nc.sync.dma_start(
                        out=wm_sb[:, wi * 1536:(wi + 1) * 1536].rearrange(
                            "p (dc j) -> p dc j", dc=6),
                        in_=wm[wi].rearrange("(dc p) j -> p dc j", p=P))
                ytr_sb = cp.tile([P, 6 * 512], DT_MLP, tag="ytr")
                nc.sync.dma_start(
                    out=ytr_sb[:, :].rearrange("p (dc n) -> p dc n", dc=6),
                    in_=ytr.rearrange("(dc p) n -> p dc n", p=P))
                xtr_sb = cp.tile([P, 6 * 512], DT_MLP, tag="xtr")
                nc.sync.dma_start(
                    out=xtr_sb[:, :].rearrange("p (dc n) -> p dc n", dc=6),
                    in_=xtr.rearrange("(dc p) n -> p dc n", p=P))
                uvw_sb = cp.tile([P, 4 * 2 * KH], DT_MAIN, tag="uvw")  # ((wi*2+jc)*7 + kk)
                nc.sync.dma_start(
                    out=uvw_sb[:, :].rearrange("p (w jc k) -> p w jc k", w=4, jc=2),
                    in_=uvw.rearrange("w (jc p) k -> p w jc k", p=P))
                bias_sb = cp.tile([P, 8], F32, tag="bias")            # (wi*2 + jc)
                nc.sync.dma_start(
                    out=bias_sb[:, :].rearrange("p (w jc) -> p w jc", w=4),
                    in_=biasv.rearrange("w (jc p) -> p w jc", p=P))
                id_sb = cp.tile([P, P], BF16, tag="id")
                nc.sync.dma_start(out=id_sb[:, :], in_=ident)
                a_sb = cp.tile([P, KH * 2 * 2 * 128], DT_MAIN, tag="a")  # (((kk*2+ic)*2+jh)*128+c)
                nc.sync.dma_start(
                    out=a_sb[:, :].rearrange("p (k ic jh c) -> p k ic jh c", k=KH, ic=2, jh=2),
                    in_=abig.rearrange("k (ic p) (jh c) -> p k ic jh c", p=P, c=P))

                hT_sb = cp.tile([P, 2 * 256], DT_MAIN, tag="hT")      # (jc*256 + m)
                tT_sb = cp.tile([P, 2 * 512], DT_MAIN, tag="tT")      # (jc*512 + n')
                headT_sb = cp.tile([P, 2 * 256], DT_MAIN, tag="headT")
                tailT_sb = cp.tile([P, 2 * 512], DT_MAIN, tag="tailT")

                # ---- stage A: the four MLPs, transposed layout ----
                def mlp(wi, rhs_sb, nfree, dst_sb, func, alpha):
                    for jc in range(2):
                        pt = psa.tile([P, 512], mybir.dt.float32, tag="tmp")
                        for dc in range(6):
                            nc.tensor.matmul(
                                pt[:, :nfree],
                                wm_sb[:, (wi * 6 + dc) * 256 + jc * 128:(wi * 6 + dc) * 256 + (jc + 1) * 128],
                                rhs_sb[:, dc * nfree:(dc + 1) * nfree],
                                start=(dc == 0), stop=(dc == 5))
                        nc.scalar.activation(
                            dst_sb[:, jc * nfree:(jc + 1) * nfree], pt[:, :nfree],
                            func, bias=bias_sb[:, wi * 2 + jc:wi * 2 + jc + 1], alpha=alpha)

                mlp(0, xtc_sb, 256, hT_sb, GELU, 0.0)
                mlp(1, ytr_sb, 512, tT_sb, GELU, 0.0)
                mlp(2, xtc_sb, 256, headT_sb, LRELU, 0.01)
                mlp(3, xtr_sb, 512, tailT_sb, LRELU, 0.01)

                # ---- stage B: rank-1 row/col terms ----
                rt_ps = psa.tile([KH, 256], mybir.dt.float32, tag="tmp")
                for si, (ui, src, nf) in enumerate([(0, hT_sb, 256), (1, headT_sb, 256)]):
                    for jc in range(2):
                        nc.tensor.matmul(
                            rt_ps[:, :],
                            uvw_sb[:, (ui * 2 + jc) * KH:(ui * 2 + jc + 1) * KH],
                            src[:, jc * nf:(jc + 1) * nf],
                            start=(si == 0 and jc == 0), stop=(si == 1 and jc == 1))
                cv_ps = psa.tile([KH, 512], mybir.dt.float32, tag="tmp")
                for si, (ui, src, nf) in enumerate([(2, tT_sb, 512), (3, tailT_sb, 512)]):
                    for jc in range(2):
                        nc.tensor.matmul(
                            cv_ps[:, :],
                            uvw_sb[:, (ui * 2 + jc) * KH:(ui * 2 + jc + 1) * KH],
                            src[:, jc * nf:(jc + 1) * nf],
                            start=(si == 0 and jc == 0), stop=(si == 1 and jc == 1))
                rt_sb = wp.tile([KH, 256], DT_MAIN, tag="rt")
                cv_sb = wp.tile([KH, 512], DT_MAIN, tag="cv")
                nc.vector.tensor_copy(rt_sb[:, :], rt_ps[:, :])
                nc.vector.tensor_copy(cv_sb[:, :], cv_ps[:, :])

                # DRAM round-trip: pack [r; ones] and [ones; cv] as 2-row operands
                scr_r = dp.tile([2, KH * 256], DT_MAIN, tag="scr_r")
                scr_c = dp.tile([2, KH * 512], DT_MAIN, tag="scr_c")
                nc.sync.dma_start(out=scr_r[0:1, :], in_=rt_sb[:, :])
                nc.sync.dma_start(out=scr_r[1:2, :], in_=onesr[:, :KH * 256])
                nc.sync.dma_start(out=scr_c[0:1, :], in_=onesr[:, :])
                nc.sync.dma_start(out=scr_c[1:2, :], in_=cv_sb[:, :])
                rall = cp.tile([2, KH * 256], DT_MAIN, tag="rall")    # row0 = r, row1 = ones
                cvall = cp.tile([2, KH * 512], DT_MAIN, tag="cvall")  # row0 = ones, row1 = cv
                nc.sync.dma_start(out=rall[0:2, :], in_=scr_r[:, :])
                nc.sync.dma_start(out=cvall[0:2, :], in_=scr_c[:, :])

                # ---- stage C: per-class biaffine ----
                import concourse.bass as bass_mod
                for kk in range(KH):
                    g_ps = psa.tile([P, 512], mybir.dt.float32, tag="g")
                    for jh in range(2):
                        for ic in range(2):
                            nc.tensor.matmul(
                                g_ps[:, jh * 256:(jh + 1) * 256],
                                a_sb[:, ((kk * 2 + ic) * 2 + jh) * 128:((kk * 2 + ic) * 2 + jh + 1) * 128],
                                hT_sb[:, ic * 256:(ic + 1) * 256],
                                start=(ic == 0), stop=(ic == 1))
                    g_sb = wp.tile([P, 512], DT_MAIN, tag="gsb")
                    nc.vector.tensor_copy(g_sb[:, :], g_ps[:, :])
                    tlb = wp.tile([P, 640], BF16, tag="tl")   # tlb[p,w] = hh[kk, p+w]
                    hap = bass_mod.AP(tensor=hh.tensor, offset=kk * 767,
                                      ap=[[1, P], [1, 640]])
                    nc.sync.dma_start(out=tlb[:, :], in_=hap)
                    for mt in range(2):
                        m_ps = psm.tile([P, 512], mybir.dt.float32, tag="m")
                        for jh in range(2):
                            nc.tensor.matmul(
                                m_ps[:, :],
                                g_sb[:, jh * 256 + mt * 128:jh * 256 + (mt + 1) * 128],
                                tT_sb[:, jh * 512:(jh + 1) * 512],
                                start=(jh == 0), stop=False)
                        nc.tensor.matmul(
                            m_ps[:, :],
                            rall[:, kk * 256 + mt * 128:kk * 256 + (mt + 1) * 128],
                            cvall[:, kk * 512:(kk + 1) * 512],
                            start=False, stop=False)
                        nc.tensor.matmul(m_ps[:, :], id_sb[:, :],
                                         tlb[:, mt * 128:mt * 128 + 512],
                                         start=False, stop=True)
                        osb = wp.tile([P, 512], F32, tag="osb")
                        if mt == 0:
                            nc.scalar.copy(osb[:, :], m_ps[:, :])
                        else:
                            nc.vector.tensor_copy(osb[:, :], m_ps[:, :])
                        nc.sync.dma_start(out=out_d[kk, mt], in_=osb[:, :])

            if reps:
                with tc.For_i(0, reps, 1) as iv:
                    body(iv)
            else:
                body()

    nc.compile()
    return nc


def _get_program(act_mode="hw", reps=0, mlp_dt=None, main_dt=None):
    key = (act_mode, reps, mlp_dt or MLP_DT, main_dt or MAIN_DT)
    if key not in _nc:
        _nc[key] = _build_program(act_mode, reps, mlp_dt, main_dt)
    return _nc[key]


def _cast(a, dt_name):
    if dt_name == "f32r":
        return np.ascontiguousarray(a, dtype=np.float32)
    import ml_dtypes
    return np.ascontiguousarray(np.asarray(a, np.float32).astype(ml_dtypes.bfloat16))


def make_in_maps(x, y, mlp1_w, mlp1_b, mlp2_w, mlp2_b, head_w, head_b,
                 tail_w, tail_b, biaf_W, W, size_emb, mlp_dt=None, main_dt=None):
    import ml_dtypes
    mlp_dt = mlp_dt or MLP_DT
    main_dt = main_dt or MAIN_DT
    f = np.float32
    x = np.asarray(x, f)
    y = np.asarray(y, f)
    wm_all = _cast(np.stack([mlp1_w.T, mlp2_w.T, head_w.T, tail_w.T]), mlp_dt)
    biasv = np.ascontiguousarray(
        np.stack([mlp1_b, mlp2_b, head_b, tail_b]).astype(f))          # (4,256)

    Ws = W[:, 514:539]
    cval = W[:, 256] + W[:, 513] + biaf_W[:, 256, 256]                 # (14,)
    T0 = (size_emb @ Ws.T).T + cval[:, None]                           # (14,30)

    xT = {b: np.ascontiguousarray(x[b].T) for b in range(B)}           # (768,512)
    xTr = {b: x[b].T[:, ::-1] for b in range(B)}
    yTr = {b: y[b].T[:, ::-1] for b in range(B)}
    identm = np.eye(P, dtype=f).astype(ml_dtypes.bfloat16)
    onesr = _cast(np.ones((1, KH * 512), f), main_dt)

    in_maps = []
    for c in range(8):
        b, mh, khalf = c // 4, (c // 2) % 2, c % 2
        m0, k0 = mh * 256, khalf * KH
        ks = slice(k0, k0 + KH)
        uvw_m = _cast(np.stack([
            biaf_W[ks, :256, 256].T,     # U    (256,7)
            W[ks, :256].T,               # WhT
            biaf_W[ks, 256, :256].T,     # V
            W[ks, 257:513].T,            # WtT
        ]), main_dt)                                                    # (4,256,7)
        wprime = np.arange(767)
        hh_m = np.ascontiguousarray(np.stack(
            [T0[k0 + kk][np.clip(526 - (wprime + m0), 0, 29)] for kk in range(KH)]
        ).astype(np.float32).astype(ml_dtypes.bfloat16))                # (7,767) bf16
        in_maps.append({
            "wm": wm_all,
            "xtc": _cast(xT[b][:, m0:m0 + 256], mlp_dt),
            "ytr": _cast(yTr[b], mlp_dt),
            "xtr": _cast(xTr[b], mlp_dt),
            "abig": _cast(biaf_W[ks, :256, :256], main_dt),
            "uvw": uvw_m,
            "biasv": biasv,
            "hh": hh_m,
            "ident": identm,
            "onesr": onesr,
        })
    return in_maps


def assemble(results):
    out = np.empty((B, CLS, N, N), np.float32)
    for c, r in enumerate(results):
        b, mh, khalf = c // 4, (c // 2) % 2, c % 2
        blk = r["out"].reshape(KH, 256, 512)[:, :, ::-1]   # un-flip n
        out[b, khalf * KH:(khalf + 1) * KH, mh * 256:(mh + 1) * 256, :] = blk
    return out


def kernel(**inputs):
    from concourse import bass_utils
    nc = _get_program()
    in_maps = make_in_maps(
        inputs["x"], inputs["y"],
        inputs["mlp1_w"], inputs["mlp1_b"], inputs["mlp2_w"], inputs["mlp2_b"],
        inputs["head_w"], inputs["head_b"], inputs["tail_w"], inputs["tail_b"],
        inputs["biaf_W"], inputs["W"], inputs["size_emb"])
    res = bass_utils.run_bass_kernel_spmd(nc, in_maps, core_ids=list(range(8)),
                                          trace=False)
    return assemble(res.results)
